# revision 11
# baseline (speedup 1.0000x reference)
"""Trainium2 Bass kernel for a 20-layer LSTM encoder (nn_EncounterAutoencoder).

Reference computation (per PyTorch LSTM semantics, fp32):
  20 stacked LSTM layers, H=128, E=768, B=64, T=512.
  Output = hidden state of layer 19 at t=511  ->  [64, 128].

Sharding: data-parallel over batch (64 -> 8 per core, 8 cores), weights
replicated. Per core we run a layer-wavefront: at step s, layer l processes
timestep t = s - l, so all 20 layers are in flight at once (531 steps).

Per-core layout:
  - Layers grouped in 5 "quads" of 4.  Gate pre-activations for quad q live in
    one PSUM bank [128, 512]: partitions 32j..32j+8 hold layer 4q+j's batch
    rows, free dim = 512 gate units (order i,f,o,g after host-side reorder so
    sigmoid gates are contiguous).
  - Per step+layer, three col-tiled matmuls accumulate into that bank:
      Whh^T stream x h-stationary, Wih^T stream x y-stationary, rank-1 bias.
    Stationaries are tiny (8 cols) so weights ride the fast rhs-stream path.
  - h must re-enter the next matmul H-major; a PE transpose per quad
    ([128,128], batch-major -> H-major) + DVE evac produces hT (double
    buffered by step parity).
  - Layer 0's input projection (contraction E=768) is precomputed as a bulk
    matmul into DRAM and streamed back one [8, 512] slice per step.
"""

import numpy as np
from contextlib import ExitStack

import concourse.bass as bass
import concourse.mybir as mybir
import concourse.tile as tile
from concourse import bacc
from concourse import bass_utils
from concourse.masks import make_identity

H = 128
E = 768
L = 20
NCORES = 8
FULL_B = 64
FULL_T = 512
BL = FULL_B // NCORES  # 8 batch rows per core
G = 4 * H  # 512 gate units per layer
NQ = 5  # 5 quads of 4 layers

FP = mybir.dt.float32
FPR = mybir.dt.float32r
AF = mybir.ActivationFunctionType

# gate block permutation: torch order [i, f, g, o] -> kernel order [i, f, o, g]
GATE_PERM = [0, 1, 3, 2]


def _reorder_gates(w):
    # w: [4H, ...] -> permute 128-row blocks
    blocks = [w[g * H:(g + 1) * H] for g in GATE_PERM]
    return np.concatenate(blocks, axis=0)


def build(nc: bass.Bass, T: int):
    """Emit the kernel IR for sequence length T (T=FULL_T for real runs)."""
    NSTEP = T + L - 1
    RT = (T * BL) // 128  # row-tiles for the bulk layer-0 projection
    assert (T * BL) % 128 == 0

    # ---- DRAM I/O ----
    xT = nc.dram_tensor("xT", [E, T * BL], FPR, kind="ExternalInput").ap()
    whhT_d = nc.dram_tensor("whhT", [H, L, G], FP, kind="ExternalInput").ap()
    wihT_d = nc.dram_tensor("wihT", [H, L - 1, G], FP, kind="ExternalInput").ap()
    wih0T_d = nc.dram_tensor("wih0T", [E, G], FPR, kind="ExternalInput").ap()
    bias_d = nc.dram_tensor("biases", [L - 1, G], FP, kind="ExternalInput").ap()
    bias0_d = nc.dram_tensor("bias0", [G], FP, kind="ExternalInput").ap()
    out_d = nc.dram_tensor("out", [BL, H], FP, kind="ExternalOutput").ap()
    pre0_d = nc.dram_tensor("pre0", [T * BL, G], FP, kind="Internal").ap()

    with tile.TileContext(nc) as tc, ExitStack() as ctx:
        const = ctx.enter_context(tc.tile_pool(name="const", bufs=1))
        state = ctx.enter_context(tc.tile_pool(name="state", bufs=1))
        psum = ctx.enter_context(tc.tile_pool(name="psum", bufs=1, space="PSUM"))
        work = ctx.enter_context(tc.tile_pool(name="work", bufs=2))
        p0pool = ctx.enter_context(tc.tile_pool(name="p0pool", bufs=3))
        xpool = ctx.enter_context(tc.tile_pool(name="xpool", bufs=3))
        b_ps_pool = ctx.enter_context(tc.tile_pool(name="bps", bufs=1, space="PSUM"))

        # ---- persistent SBUF ----
        whh = const.tile([H, L, G], FP, tag="whh")
        wih = const.tile([H, L - 1, G], FP, tag="wih")
        wih0 = const.tile([128, E // 128, G], FPR, tag="wih0")
        biases = const.tile([1, L - 1, G], FP, tag="biases")
        bias0b = const.tile([128, G], FP, tag="bias0b")
        ones1 = const.tile([1, 32], FP, tag="ones1")
        ident = const.tile([128, 128], FP, tag="ident")

        c = state.tile([128, NQ, H], FP, tag="c")
        hT = state.tile([H, 2, NQ, 128], FP, tag="hT")

        gates_ps = psum.tile([128, NQ, G], FP, tag="gates")   # 5 banks
        tp_ps = psum.tile([128, NQ, H], FP, tag="tp")         # 2 banks

        # ---- load constants ----
        nc.sync.dma_start(out=whh, in_=whhT_d)
        nc.sync.dma_start(out=wih, in_=wihT_d)
        nc.sync.dma_start(out=wih0, in_=wih0T_d.rearrange("(k p) g -> p k g", p=128))
        nc.sync.dma_start(out=biases, in_=bias_d[None])
        bias0_bcast = bass.AP(tensor=bias0_d.tensor, offset=0, ap=[[0, 128], [1, G]])
        nc.gpsimd.dma_start(out=bias0b, in_=bias0_bcast)
        nc.vector.memset(ones1, 1.0)
        make_identity(nc, ident)
        nc.vector.memset(c, 0.0)
        nc.vector.memset(hT, 0.0)
        nc.vector.memset(gates_ps, 0.0)

        # ---- phase B: bulk layer-0 input projection -> pre0_d ----
        # pre0[t*BL+b, :] = x[b,t,:] @ Wih0^T + bias0   (xT[e, t*BL+b] layout)
        for rt in range(RT):
            ps = b_ps_pool.tile([128, G], FP, tag="p0ps")
            for k in range(E // 128):
                xk = xpool.tile([128, 128], FPR, tag="xk")
                nc.sync.dma_start(
                    out=xk, in_=xT[k * 128:(k + 1) * 128, rt * 128:(rt + 1) * 128]
                )
                nc.tensor.matmul(
                    ps,
                    xk,
                    wih0[:, k, :],
                    start=(k == 0),
                    stop=(k == E // 128 - 1),
                )
            u = xpool.tile([128, G], FP, tag="p0u")
            nc.vector.tensor_add(u, ps, bias0b)
            nc.sync.dma_start(out=pre0_d[rt * 128:(rt + 1) * 128, :], in_=u)

        # ---- phase C: the wavefront ----
        def emit_step(parity, lmin, lmax, pre0_row):
            """One wavefront step.

            pre0_row: None (skip layer-0 addend, tail) or an int / scalar
            expression giving the row offset (t*BL) into pre0_d.
            Returns the batch-major h tile (for the final output DMA).
            """
            hT_rd = hT[:, parity]
            hT_wr = hT[:, 1 - parity]

            for q in range(NQ):
                for j in range(4):
                    l = 4 * q + j
                    if l < lmin or l > lmax:
                        continue
                    out_ps = gates_ps[32 * j:32 * (j + 1), q, :]
                    tp = (0, 32 * j)
                    nc.tensor.matmul(
                        out_ps,
                        hT_rd[:, q, 32 * j:32 * (j + 1)],
                        whh[:, l, :],
                        start=True,
                        stop=(l == 0),
                        tile_position=tp,
                    )
                    if l > 0:
                        lq, lj = divmod(l - 1, 4)
                        nc.tensor.matmul(
                            out_ps,
                            hT_rd[:, lq, 32 * lj:32 * (lj + 1)],
                            wih[:, l - 1, :],
                            start=False,
                            stop=False,
                            tile_position=tp,
                        )
                        nc.tensor.matmul(
                            out_ps,
                            ones1,
                            biases[:, l - 1, :],
                            start=False,
                            stop=True,
                            tile_position=tp,
                        )

            if pre0_row is not None:
                p0 = p0pool.tile([BL, G], FP, tag="p0t")
                if isinstance(pre0_row, int):
                    nc.sync.dma_start(out=p0, in_=pre0_d[pre0_row:pre0_row + BL, :])
                else:
                    nc.sync.dma_start(out=p0, in_=pre0_d[bass.ds(pre0_row, BL), :])
                nc.vector.tensor_add(gates_ps[0:BL, 0, :], gates_ps[0:BL, 0, :], p0)

            sig = work.tile([128, NQ, 3 * H], FP, tag="sig")
            nc.scalar.activation(sig, gates_ps[:, :, 0:3 * H], AF.Sigmoid)
            tg = work.tile([128, NQ, H], FP, tag="tg")
            nc.scalar.activation(tg, gates_ps[:, :, 3 * H:4 * H], AF.Tanh)

            ig = work.tile([128, NQ, H], FP, tag="ig")
            nc.gpsimd.tensor_mul(ig, sig[:, :, 0:H], tg)
            fc = work.tile([128, NQ, H], FP, tag="fc")
            nc.vector.tensor_mul(fc, sig[:, :, H:2 * H], c)
            nc.vector.tensor_add(c, fc, ig)
            tcn = work.tile([128, NQ, H], FP, tag="tcn")
            nc.scalar.activation(tcn, c, AF.Tanh)
            hbm = work.tile([128, NQ, H], FP, tag="hbm")
            nc.gpsimd.tensor_mul(hbm, sig[:, :, 2 * H:3 * H], tcn)

            for q in range(NQ):
                nc.tensor.transpose(tp_ps[:, q, :], hbm[:, q, :], ident)
                nc.vector.tensor_copy(hT_wr[:, q, :], tp_ps[:, q, :])
            return hbm

        # head: layers ramp in; static pre0 offsets
        for s in range(min(L, T)):
            emit_step(s % 2, 0, s, s * BL)

        # middle: full rectangle, hardware loop. UNROLL must be even (step
        # parity is baked per unroll instance) and divide T-L exactly.
        if T > L:
            n_mid = T - L
            UNROLL = 12
            while UNROLL > 2 and (n_mid % UNROLL != 0 or UNROLL % 2 != 0):
                UNROLL -= 2
            assert n_mid % UNROLL == 0 and UNROLL % 2 == 0

            def mid_body(iv0, unroll):
                for k in range(unroll):
                    emit_step((L + k) % 2, 0, L - 1, iv0 * BL + k * BL)

            tc.For_i_unrolled_general(
                start=L,
                end=T,
                step=1,
                unrollable_body=mid_body,
                max_unroll=UNROLL,
                hint_engines=(mybir.EngineType.PE,),
            )

        # tail: layers ramp out; no layer-0 input left
        hbm_last = None
        for s in range(T, NSTEP):
            hbm_last = emit_step(s % 2, s - (T - 1), L - 1, None)

        if hbm_last is None:  # T <= L edge (tiny sim configs)
            hbm_last = emit_step(NSTEP % 2, 0, L - 1, None)

        nc.sync.dma_start(out=out_d, in_=hbm_last[96:96 + BL, NQ - 1, :])

    return nc


def prep_inputs(x, Wih0, Whh0, bih0, bhh0, Wih, Whh, bih, bhh):
    """Host-side: gate-reorder weights, transpose for the device layouts,
    shard x by batch. Returns (in_maps, T)."""
    B, T, _ = x.shape
    whhT = np.empty((H, L, G), np.float32)
    wihT = np.empty((H, L - 1, G), np.float32)
    biases = np.empty((L - 1, G), np.float32)
    whhT[:, 0, :] = _reorder_gates(np.asarray(Whh0)).T
    for l in range(1, L):
        whhT[:, l, :] = _reorder_gates(np.asarray(Whh[l - 1])).T
        wihT[:, l - 1, :] = _reorder_gates(np.asarray(Wih[l - 1])).T
        biases[l - 1] = _reorder_gates(np.asarray(bih[l - 1]) + np.asarray(bhh[l - 1]))
    wih0T = np.ascontiguousarray(_reorder_gates(np.asarray(Wih0)).T)  # [E, G]
    bias0 = _reorder_gates(np.asarray(bih0) + np.asarray(bhh0))

    in_maps = []
    for core in range(NCORES):
        xs = np.asarray(x[core * BL:(core + 1) * BL])  # [BL, T, E]
        xT = np.ascontiguousarray(np.transpose(xs, (2, 1, 0)).reshape(E, T * BL))
        in_maps.append(
            {
                "xT": xT,
                "whhT": whhT,
                "wihT": wihT,
                "wih0T": wih0T,
                "biases": biases,
                "bias0": np.ascontiguousarray(bias0),
            }
        )
    return in_maps


def kernel(**inputs):
    x = np.asarray(inputs["x"], np.float32)
    B, T, _ = x.shape
    assert B == FULL_B and T == FULL_T
    nc = bacc.Bacc("TRN2", target_bir_lowering=False, debug=False, num_devices=NCORES)
    build(nc, T)
    nc.compile()
    in_maps = prep_inputs(**inputs)
    res = bass_utils.run_bass_kernel_spmd(nc, in_maps, core_ids=list(range(NCORES)))
    out = np.concatenate([r["out"] for r in res.results], axis=0)
    return out.astype(np.float32)


# revision 16
# speedup vs baseline: 42.4326x; 42.4326x over previous
"""Trainium2 Bass kernel for a 20-layer LSTM encoder (nn_EncounterAutoencoder).

Reference computation (per PyTorch LSTM semantics, fp32):
  20 stacked LSTM layers, H=128, E=768, B=64, T=512.
  Output = hidden state of layer 19 at t=511  ->  [64, 128].

Sharding: data-parallel over batch (64 -> 8 per core, 8 cores), weights
replicated. Per core we run a layer-wavefront: at step s, layer l processes
timestep t = s - l, so all 20 layers are in flight at once (531 steps).

Per-core layout:
  - Layers grouped in 5 "quads" of 4.  Gate pre-activations for quad q live in
    one PSUM bank [128, 512]: partitions 32j..32j+8 hold layer 4q+j's batch
    rows, free dim = 512 gate units (order i,f,o,g after host-side reorder so
    sigmoid gates are contiguous).
  - Per step+layer, three col-tiled matmuls accumulate into that bank:
      Whh^T stream x h-stationary, Wih^T stream x y-stationary, rank-1 bias.
    Stationaries are tiny (8 cols) so weights ride the fast rhs-stream path.
  - h must re-enter the next matmul H-major; a PE transpose per quad
    ([128,128], batch-major -> H-major) + DVE evac produces hT (double
    buffered by step parity).
  - Layer 0's input projection (contraction E=768) is precomputed as a bulk
    matmul into DRAM and streamed back one [8, 512] slice per step.
"""

import numpy as np
import ml_dtypes
from contextlib import ExitStack

import concourse.bass as bass
import concourse.mybir as mybir
import concourse.tile as tile
from concourse import bacc
from concourse import bass_utils
from concourse.masks import make_identity

H = 128
E = 768
L = 20
NCORES = 8
FULL_B = 64
FULL_T = 512
BL = FULL_B // NCORES  # 8 batch rows per core
G = 4 * H  # 512 gate units per layer
NQ = 5  # 5 quads of 4 layers

FP = mybir.dt.float32
FPR = mybir.dt.float32r
BF = mybir.dt.bfloat16
AF = mybir.ActivationFunctionType

# gate block permutation: torch order [i, f, g, o] -> kernel order [i, f, o, g]
GATE_PERM = [0, 1, 3, 2]


def _reorder_gates(w):
    # w: [4H, ...] -> permute 128-row blocks
    blocks = [w[g * H:(g + 1) * H] for g in GATE_PERM]
    return np.concatenate(blocks, axis=0)


def build(nc: bass.Bass, T: int, feats=frozenset({"mm", "wih", "bias", "act", "ew", "tr", "p0"})):
    """Emit the kernel IR for sequence length T (T=FULL_T for real runs)."""
    NSTEP = T + L - 1
    RT = (T * BL) // 128  # row-tiles for the bulk layer-0 projection
    assert (T * BL) % 128 == 0

    # ---- DRAM I/O ----
    xT = nc.dram_tensor("xT", [E, T * BL], FPR, kind="ExternalInput").ap()
    whhT_d = nc.dram_tensor("whhT", [H, L, G], BF, kind="ExternalInput").ap()
    wihT_d = nc.dram_tensor("wihT", [H, L - 1, G], BF, kind="ExternalInput").ap()
    wih0T_d = nc.dram_tensor("wih0T", [E, G], FPR, kind="ExternalInput").ap()
    bias_d = nc.dram_tensor("biases", [L - 1, G], BF, kind="ExternalInput").ap()
    bias0_d = nc.dram_tensor("bias0", [G], FP, kind="ExternalInput").ap()
    out_d = nc.dram_tensor("out", [BL, H], FP, kind="ExternalOutput").ap()
    pre0_d = nc.dram_tensor("pre0", [T * BL, G], FP, kind="Internal").ap()

    with tile.TileContext(nc) as tc, ExitStack() as ctx:
        const = ctx.enter_context(tc.tile_pool(name="const", bufs=1))
        state = ctx.enter_context(tc.tile_pool(name="state", bufs=1))
        psum = ctx.enter_context(tc.tile_pool(name="psum", bufs=1, space="PSUM"))
        work = ctx.enter_context(tc.tile_pool(name="work", bufs=2))
        p0pool = ctx.enter_context(tc.tile_pool(name="p0pool", bufs=3))
        xpool = ctx.enter_context(tc.tile_pool(name="xpool", bufs=3))
        b_ps_pool = ctx.enter_context(tc.tile_pool(name="bps", bufs=1, space="PSUM"))

        # ---- persistent SBUF ----
        whh = const.tile([H, L, G], BF, tag="whh")
        wih = const.tile([H, L - 1, G], BF, tag="wih")
        wih0 = const.tile([128, E // 128, G], FPR, tag="wih0")
        biases = const.tile([1, L - 1, G], BF, tag="biases")
        bias0b = const.tile([128, G], FP, tag="bias0b")
        ones1 = const.tile([1, 32], BF, tag="ones1")
        ident = const.tile([128, 128], BF, tag="ident")

        c = state.tile([128, NQ, H], FP, tag="c")
        hT = state.tile([H, 2, NQ, 128], BF, tag="hT")

        gates_ps = psum.tile([128, NQ, G], FP, tag="gates")   # 5 banks
        tp_ps = psum.tile([128, NQ, H], BF, tag="tp")         # 2 banks

        # ---- load constants ----
        nc.sync.dma_start(out=whh, in_=whhT_d)
        nc.sync.dma_start(out=wih, in_=wihT_d)
        nc.sync.dma_start(out=wih0, in_=wih0T_d.rearrange("(k p) g -> p k g", p=128))
        nc.sync.dma_start(out=biases, in_=bias_d[None])
        bias0_bcast = bass.AP(tensor=bias0_d.tensor, offset=0, ap=[[0, 128], [1, G]])
        nc.gpsimd.dma_start(out=bias0b, in_=bias0_bcast)
        nc.vector.memset(ones1, 1.0)
        make_identity(nc, ident)
        nc.vector.memset(c, 0.0)
        nc.vector.memset(hT, 0.0)
        nc.vector.memset(gates_ps, 0.0)

        # ---- phase B: bulk layer-0 input projection -> pre0_d ----
        # pre0[t*BL+b, :] = x[b,t,:] @ Wih0^T + bias0   (xT[e, t*BL+b] layout)
        for rt in range(RT):
            ps = b_ps_pool.tile([128, G], FP, tag="p0ps")
            for k in range(E // 128):
                xk = xpool.tile([128, 128], FPR, tag="xk")
                nc.sync.dma_start(
                    out=xk, in_=xT[k * 128:(k + 1) * 128, rt * 128:(rt + 1) * 128]
                )
                nc.tensor.matmul(
                    ps,
                    xk,
                    wih0[:, k, :],
                    start=(k == 0),
                    stop=(k == E // 128 - 1),
                )
            u = xpool.tile([128, G], FP, tag="p0u")
            nc.vector.tensor_add(u, ps, bias0b)
            nc.sync.dma_start(out=pre0_d[rt * 128:(rt + 1) * 128, :], in_=u)

        # ---- phase C: the wavefront ----
        def emit_step(parity, lmin, lmax, pre0_row):
            """One wavefront step.

            pre0_row: None (skip layer-0 addend, tail) or an int / scalar
            expression giving the row offset (t*BL) into pre0_d.
            Returns the batch-major h tile (for the final output DMA).
            """
            hT_rd = hT[:, parity]
            hT_wr = hT[:, 1 - parity]

            for q in range(NQ):
                for j in range(4):
                    l = 4 * q + j
                    if l < lmin or l > lmax:
                        continue
                    out_ps = gates_ps[32 * j:32 * (j + 1), q, :]
                    tp = (0, 32 * j)
                    if "mm" not in feats:
                        continue
                    nc.tensor.matmul(
                        out_ps,
                        hT_rd[:, q, 32 * j:32 * (j + 1)],
                        whh[:, l, :],
                        start=True,
                        stop=(l == 0),
                        tile_position=tp,
                    )
                    if l > 0 and "wih" in feats:
                        lq, lj = divmod(l - 1, 4)
                        nc.tensor.matmul(
                            out_ps,
                            hT_rd[:, lq, 32 * lj:32 * (lj + 1)],
                            wih[:, l - 1, :],
                            start=False,
                            stop=("bias" not in feats),
                            tile_position=tp,
                        )
                        if "bias" in feats:
                            nc.tensor.matmul(
                                out_ps,
                                ones1,
                                biases[:, l - 1, :],
                                start=False,
                                stop=True,
                                tile_position=tp,
                            )

            if pre0_row is not None and "p0" in feats:
                p0 = p0pool.tile([BL, G], FP, tag="p0t")
                if isinstance(pre0_row, int):
                    nc.sync.dma_start(out=p0, in_=pre0_d[pre0_row:pre0_row + BL, :])
                else:
                    nc.sync.dma_start(out=p0, in_=pre0_d[bass.ds(pre0_row, BL), :])
                nc.vector.tensor_add(gates_ps[0:BL, 0, :], gates_ps[0:BL, 0, :], p0)

            sig = work.tile([128, NQ, 3 * H], FP, tag="sig")
            tg = work.tile([128, NQ, H], FP, tag="tg")
            if "act" in feats:
                for q in range(NQ):
                    nc.scalar.activation(sig[:, q, :], gates_ps[:, q, 0:3 * H],
                                         AF.Sigmoid)
                    nc.scalar.activation(tg[:, q, :], gates_ps[:, q, 3 * H:4 * H],
                                         AF.Tanh)
            else:
                nc.vector.memset(sig, 0.5)
                nc.vector.memset(tg, 0.1)

            hbm = work.tile([128, NQ, H], BF, tag="hbm")
            if "ew" in feats:
                ig = work.tile([128, NQ, H], FP, tag="ig")
                nc.gpsimd.tensor_mul(ig, sig[:, :, 0:H], tg)
                fc = work.tile([128, NQ, H], FP, tag="fc")
                nc.vector.tensor_mul(fc, sig[:, :, H:2 * H], c)
                nc.vector.tensor_add(c, fc, ig)
                tcn = work.tile([128, NQ, H], FP, tag="tcn")
                nc.scalar.activation(tcn, c, AF.Tanh)
                nc.gpsimd.tensor_mul(hbm, sig[:, :, 2 * H:3 * H], tcn)
            else:
                tcn = None
                nc.vector.tensor_copy(hbm, sig[:, :, 0:H])

            for q in range(NQ):
                if "tr" in feats:
                    nc.tensor.transpose(tp_ps[:, q, :], hbm[:, q, :], ident)
                    nc.vector.tensor_copy(hT_wr[:, q, :], tp_ps[:, q, :])
                else:
                    nc.vector.tensor_copy(hT_wr[:, q, 0:BL], hbm[0:BL, q, 0:BL])
            return hbm, sig, tcn

        # head: layers ramp in; static pre0 offsets
        for s in range(min(L, T)):
            emit_step(s % 2, 0, s, s * BL)

        # middle: full rectangle, hardware loop. UNROLL must be even (step
        # parity is baked per unroll instance) and divide T-L exactly.
        if T > L:
            n_mid = T - L
            UNROLL = 12
            while UNROLL > 2 and (n_mid % UNROLL != 0 or UNROLL % 2 != 0):
                UNROLL -= 2
            assert n_mid % UNROLL == 0 and UNROLL % 2 == 0

            def mid_body(iv0, unroll):
                for k in range(unroll):
                    emit_step((L + k) % 2, 0, L - 1, iv0 * BL + k * BL)

            tc.For_i_unrolled_general(
                start=L,
                end=T,
                step=1,
                unrollable_body=mid_body,
                max_unroll=UNROLL,
                hint_engines=(mybir.EngineType.PE,),
            )

        # tail: layers ramp out; no layer-0 input left
        last = None
        for s in range(T, NSTEP):
            last = emit_step(s % 2, s - (T - 1), L - 1, None)

        if last is None:  # T <= L edge (tiny sim configs)
            last = emit_step(NSTEP % 2, 0, L - 1, None)

        _, sig_l, tcn_l = last
        hout = state.tile([BL, H], FP, tag="hout")
        nc.vector.tensor_mul(
            hout,
            sig_l[96:96 + BL, NQ - 1, 2 * H:3 * H],
            tcn_l[96:96 + BL, NQ - 1, :],
        )
        nc.sync.dma_start(out=out_d, in_=hout)

    return nc


def prep_inputs(x, Wih0, Whh0, bih0, bhh0, Wih, Whh, bih, bhh):
    """Host-side: gate-reorder weights, transpose for the device layouts,
    shard x by batch. Returns (in_maps, T)."""
    B, T, _ = x.shape
    whhT = np.empty((H, L, G), ml_dtypes.bfloat16)
    wihT = np.empty((H, L - 1, G), ml_dtypes.bfloat16)
    biases = np.empty((L - 1, G), ml_dtypes.bfloat16)
    whhT[:, 0, :] = _reorder_gates(np.asarray(Whh0)).T
    for l in range(1, L):
        whhT[:, l, :] = _reorder_gates(np.asarray(Whh[l - 1])).T
        wihT[:, l - 1, :] = _reorder_gates(np.asarray(Wih[l - 1])).T
        biases[l - 1] = _reorder_gates(np.asarray(bih[l - 1]) + np.asarray(bhh[l - 1]))
    wih0T = np.ascontiguousarray(_reorder_gates(np.asarray(Wih0)).T)  # [E, G]
    bias0 = _reorder_gates(np.asarray(bih0) + np.asarray(bhh0))

    in_maps = []
    for core in range(NCORES):
        xs = np.asarray(x[core * BL:(core + 1) * BL])  # [BL, T, E]
        xT = np.ascontiguousarray(np.transpose(xs, (2, 1, 0)).reshape(E, T * BL))
        in_maps.append(
            {
                "xT": xT,
                "whhT": whhT,
                "wihT": wihT,
                "wih0T": wih0T,
                "biases": biases,
                "bias0": np.ascontiguousarray(bias0),
            }
        )
    return in_maps


def kernel(**inputs):
    x = np.asarray(inputs["x"], np.float32)
    B, T, _ = x.shape
    assert B == FULL_B and T == FULL_T
    nc = bacc.Bacc("TRN2", target_bir_lowering=False, debug=False, num_devices=NCORES)
    build(nc, T)
    nc.compile()
    in_maps = prep_inputs(**inputs)
    res = bass_utils.run_bass_kernel_spmd(nc, in_maps, core_ids=list(range(NCORES)))
    out = np.concatenate([r["out"] for r in res.results], axis=0)
    return out.astype(np.float32)
